# revision 34
# baseline (speedup 1.0000x reference)
"""Trainium2 Bass kernel for nn_BiLSTM pairwise-scores problem.

Math (reference):
  vec  = concat(word_emb[wi], pos_emb[pi], ext_emb[ei])          [512, 425]
  h    = concat(lstm_cell_f(vec), lstm_cell_b(vec))              [512, 200]
  cat  = [h, vec] for t <= 255 else [vec, h]                     [512, 625]
  f    = cat @ w_mlp_in.T + b_mlp_in                             [512, 400]
  out  = tanh((f[:,None,:] + f[None,:,:]) @ w_mlp_out.T + b_out) [512, 512, 42]

Two host-side algebraic folds shrink the device program:
  1. mlp_in and mlp_out are both linear, so
       (f_i + f_j) @ Wo.T + b_out = cat_i @ M + cat_j @ M + b''
     with M = W_in.T @ Wo.T  [625, 42].  Each token needs only the tiny
     g' = cat @ M + b'/row projection (b' = b_in @ Wo.T + b_out/2); the
     [625 -> 400] mlp_in stage disappears entirely.
  2. The gate biases ride a ones-row appended to vec (K=426), so the gate
     PSUM already contains w.x+b and the sigmoid/tanh activations batch
     into a few wide ACT instructions with no per-gate bias operands.

Pairwise stage: out[p, j*42+r] = g'_i[p,r] + g'_j[j,r] realized as a
single K=43 matmul per 512-col chunk: lhsT = [g'_iT rows; ones row],
rhs = [periodic identity rows; g'_j flattened row], then one Tanh ACT
per 4-chunk PSUM group, emitted as bf16 (host upcasts to f32).

Sharding: 8 cores = 2 i-halves (256 rows) x 4 j-quarters (128 cols).
Each core runs an identical (SPMD) program on a permuted 384-token slice:
cols 0:256 = its i-half tokens, cols 256:384 = its j-quarter tokens.
Both 128-row i-blocks of a core share one rhs (identity + g'_j flat), so
the identity broadcast is only [42, 5376].
"""

import os
import sys

import numpy as np

for _p in ("/opt/trn_rl_repo", "/root/.axon_site/_ro/trn_rl_repo"):
    if os.path.isdir(_p) and _p not in sys.path:
        sys.path.insert(0, _p)

import ml_dtypes  # noqa: E402

import concourse.bacc as bacc  # noqa: E402
import concourse.bass as bass  # noqa: E402
import concourse.mybir as mybir  # noqa: E402
from concourse.bass_utils import run_bass_kernel_spmd  # noqa: E402
from concourse.tile import TileContext  # noqa: E402

BF16 = mybir.dt.bfloat16
F32 = mybir.dt.float32
AF = mybir.ActivationFunctionType

SEQ = 512
NREL = 42
T = 384          # per-core tokens: 256 (i-half) + 128 (j-quarter)
NI = 256         # i tokens per core
NJ = 128         # j tokens per core
JFLAT = NJ * NREL          # 5376 = per-block output row length
NFLAT = 2 * JFLAT          # 10752 = per-core output row length
IC_PER = 8 * NREL          # 336: replication period for the identity pattern

# K-dim tiling of the 426-dim (vec + ones) feature axis
KS = [(0, 128), (128, 256), (256, 384), (384, 426)]
# gate column order in the stacked [426, 600] gate weight:
# i_f o_f i_b o_b | g_f g_b   (io block first for batched sigmoid ACT)

# ---- packed bf16 constant layout: [128, NPK] ----
_SEGS = []  # name -> (rows, col_off, width)


def _seg(name, rows, width):
    off = _SEGS[-1][2] + _SEGS[-1][3] if _SEGS else 0
    _SEGS.append((name, rows, off, width))


# Layout minimizes DMA bytes: full-128-row segments first (vt/g6 k-chunks
# 0-2 + M vec-chunks 0-2), then a 42-row block (vt3, g63, m3, ic), then a
# 100-row block (mh0, mh1).  The short blocks transfer only their live
# partition rows and ride the scalar ring, landing first — so the gate
# k-loop runs k3 before k0..k2.
# M [626, 84]: cols 0:42 i-half ordering, 42:84 j-quarter ordering; rows:
# vec chunks (426, incl b' row at 425), then h_f (100), h_b (100).
for _k in range(3):
    _seg(f"vt{_k}", 128, T)
    _seg(f"g6{_k}", 128, 600)
for _k in range(3):
    _seg(f"m{_k}", 128, 2 * NREL)
_CUT_FULL = _SEGS[-1][2] + _SEGS[-1][3]
_seg("vt3", 42, T)
_seg("g63", 42, 600)
_seg("m3", 42, 2 * NREL)
_seg("ic", NREL, IC_PER)
_CUT_42 = _SEGS[-1][2] + _SEGS[-1][3]
_seg("mh0", 100, 2 * NREL)
_seg("mh1", 100, 2 * NREL)
SEG = {s[0]: s for s in _SEGS}
NPK = _SEGS[-1][2] + _SEGS[-1][3]
_CUT_HALF = 2 * (T + 600)  # sync ring split: (vt0,g60,vt1,g61) | rest


def _build_program():
    nc = bacc.Bacc()

    pk_d = nc.dram_tensor("pk", [128, NPK], BF16, kind="ExternalInput")
    out_d = nc.dram_tensor("out", [128, NFLAT], BF16, kind="ExternalOutput")

    with TileContext(nc) as tc:
        with (
            tc.tile_pool(name="const", bufs=1) as cp,
            tc.tile_pool(name="work", bufs=1) as wp,
            tc.tile_pool(name="outp", bufs=3) as op_,
        ):
            # -------- early on-chip init (no DMA deps) --------
            wsrc = cp.tile([128, 512], BF16, tag="wsrc")
            nc.gpsimd.memset(wsrc, 0.0)
            # lhsT tiles of the pairwise matmul: rows 0:42 = g'_iT, row 42
            # = 1.0.  DVE partition base must be 32-aligned, so memset
            # 32:43 and let the later g' copy overwrite rows 32:42.
            el = []
            for b in range(2):
                e = cp.tile([NREL + 1, 128], BF16, tag=f"el{b}")
                nc.vector.memset(e[32 : NREL + 1, :], 1.0)
                el.append(e)
            # warmup activations absorb the ACT table-set loads early
            warm2 = cp.tile([1, 8], F32, tag="warm2")
            nc.scalar.activation(out=warm2, in_=wsrc[0:1, 0:8], func=AF.Sigmoid)
            nc.scalar.activation(out=warm2, in_=wsrc[0:1, 0:8], func=AF.Tanh)

            # -------- input DMAs --------
            # The 16 SDMA engines share one ~200 GB/s effective pipe; all
            # input rides the sync ring smallest-first, so the k3 gate
            # chunks unblock the PE right after the warmups.
            # landing order = gate k-order (k0, k1, k3, k2); the mh block
            # (only needed by the late catM h-matmuls) goes last
            pk = cp.tile([128, NPK], BF16, tag="pk")
            pair = T + 600
            nc.sync.dma_start(out=pk[:, 0:pair], in_=pk_d[:, 0:pair])
            nc.sync.dma_start(
                out=pk[:, pair:_CUT_HALF], in_=pk_d[:, pair:_CUT_HALF]
            )
            nc.sync.dma_start(
                out=pk[0:42, _CUT_FULL:_CUT_42], in_=pk_d[0:42, _CUT_FULL:_CUT_42]
            )
            nc.sync.dma_start(
                out=pk[:, _CUT_HALF:_CUT_FULL], in_=pk_d[:, _CUT_HALF:_CUT_FULL]
            )
            nc.sync.dma_start(
                out=pk[0:100, _CUT_42:NPK], in_=pk_d[0:100, _CUT_42:NPK]
            )

            def seg(name):
                _, rows, off, width = SEG[name]
                return pk[0:rows, off : off + width]

            vt = [seg(f"vt{k}") for k in range(4)]
            g6 = [seg(f"g6{k}") for k in range(4)]
            mm = [seg(f"m{k}") for k in range(4)] + [seg("mh0"), seg("mh1")]
            ic = seg("ic")
            KORD = (0, 1, 3, 2)  # matches the input DMA landing order

            # pairwise rhs: rows 0:42 = periodic identity, row 42 = g'_j
            # flat.  Both i-blocks share it, so only JFLAT wide.
            rr = cp.tile([NREL + 1, JFLAT], BF16, tag="rr")
            ic_rep = bass.AP(
                tensor=ic.tensor,
                offset=ic.offset,
                ap=[ic.ap[0], [0, JFLAT // IC_PER], ic.ap[1]],
            )
            nc.scalar.dma_start(out=rr[0:NREL, :], in_=ic_rep)

            with tc.tile_pool(name="psum_pre", bufs=1, space="PSUM") as pp:
                io_t = pp.tile([128, 2048], F32, tag="io")
                g_t = pp.tile([100, 1024], F32, tag="g")
                gt_t = pp.tile([NREL, NI], F32, tag="gt")
                nat_t = pp.tile([128, NREL], F32, tag="nat")

                # PE warmup: start the HAM busy-window during the DMA wait
                for _ in range(4):
                    nc.tensor.matmul(
                        io_t[:, 0:512],
                        lhsT=wsrc[:, 0:128],
                        rhs=wsrc,
                        start=True,
                        stop=True,
                    )

                # -------- LSTM gates (both dirs, f-gate skipped) --------
                # i gates in adjacent banks (0,512), o gates in (1024,1536):
                # the strided ACT APs' bounding boxes then cover only their
                # own gates, so sigmoid(i) fires as soon as i_b lands.
                # k-outer loop: each k block finishes g, then i, then o, so
                # the static schedule can't strand one gate's last chunk.
                GATES = [
                    (g_t, 400, 0),      # g_f
                    (g_t, 500, 512),    # g_b
                    (io_t, 0, 0),       # i_f
                    (io_t, 200, 512),   # i_b
                    (io_t, 100, 1024),  # o_f
                    (io_t, 300, 1536),  # o_b
                ]
                for n, k in enumerate(KORD):
                    for dst, col, ca in GATES:
                        nc.tensor.matmul(
                            dst[0:100, ca : ca + T],
                            lhsT=g6[k][:, col : col + 100],
                            rhs=vt[k],
                            start=(n == 0),
                            stop=(n == 3),
                        )

                def strided_in(tile, base, stride):
                    a = tile[0:100, base : base + stride + T]
                    return bass.AP(
                        tensor=a.tensor,
                        offset=a.offset,
                        ap=[a.ap[0], [stride, 2], [1, T]],
                    )

                def strided_out(tile):
                    a = tile[0:100, 0 : 2 * T]
                    return bass.AP(
                        tensor=a.tensor,
                        offset=a.offset,
                        ap=[a.ap[0], [T, 2], [1, T]],
                    )

                # batched activations: (f,b) pairs in one ACT each
                tgs = wp.tile([100, 2 * T], BF16, tag="tgs")
                nc.scalar.activation(
                    out=strided_out(tgs), in_=strided_in(g_t, 0, 512), func=AF.Tanh
                )
                si = wp.tile([100, 2 * T], BF16, tag="si")
                nc.scalar.activation(
                    out=strided_out(si), in_=strided_in(io_t, 0, 512), func=AF.Sigmoid
                )
                # c = sig(i) * tanh(g), both dirs in one DVE op
                cc = wp.tile([100, 2 * T], BF16, tag="cc")
                nc.vector.tensor_mul(cc, si, tgs)
                tcs = wp.tile([100, 2 * T], BF16, tag="tcs")
                nc.scalar.activation(out=tcs, in_=cc, func=AF.Tanh)
                so = wp.tile([100, 2 * T], BF16, tag="so")
                nc.scalar.activation(
                    out=strided_out(so), in_=strided_in(io_t, 1024, 512),
                    func=AF.Sigmoid,
                )
                hht = cp.tile([100, 2 * T], BF16, tag="hht")
                nc.vector.tensor_mul(hht, so, tcs)
                hh = [hht[:, 0:T], hht[:, T : 2 * T]]

                # fillers pinned into the ACT/DVE gap: keep the PE activity
                # monitor from re-throttling the clock
                for _ in range(4):
                    nc.tensor.matmul(
                        io_t[:, 0:T],
                        lhsT=si[:, 0:128],
                        rhs=si[:, 0:T],
                        start=True,
                        stop=True,
                    )

                # -------- g' = cat @ M + b': transposed for i, natural
                # for j.  cat chunks: vt0..vt3 (incl ones row), h_f, h_b.
                cat = vt + hh
                for k in range(6):
                    nc.tensor.matmul(
                        gt_t,
                        lhsT=mm[k][:, 0:NREL],
                        rhs=cat[k][:, 0:NI],
                        start=(k == 0),
                        stop=(k == 5),
                    )
                for k in range(6):
                    nc.tensor.matmul(
                        nat_t,
                        lhsT=cat[k][:, NI:T],
                        rhs=mm[k][:, NREL : 2 * NREL],
                        start=(k == 0),
                        stop=(k == 5),
                    )

                # fillers pinned on h: bridge the PE gap before pairwise
                for _ in range(4):
                    nc.tensor.matmul(
                        io_t[:, 512 : 512 + T],
                        lhsT=hht[:, 0:128],
                        rhs=hht[:, 0:T],
                        start=True,
                        stop=True,
                    )

                # el rows 0:42 <- g'_iT; natural g'_j -> flatten into rr
                for b in range(2):
                    nc.vector.tensor_copy(
                        el[b][0:NREL, :], gt_t[:, b * 128 : (b + 1) * 128]
                    )
                # flatten split across both rings: half the completion
                # latency gates the first pairwise groups
                natc = wp.tile([128, NREL], BF16, tag="natc")
                nc.vector.tensor_copy(natc, nat_t)
                nc.scalar.dma_start(
                    out=rr[NREL : NREL + 1, 0 : JFLAT // 2], in_=natc[0:64, :]
                )
                nc.sync.dma_start(
                    out=rr[NREL : NREL + 1, JFLAT // 2 : JFLAT], in_=natc[64:128, :]
                )

            # -------- pairwise: tanh(g'_i + g'_j), bf16 out --------
            # per i-block: 5376 cols = chunks of 512 (+ one 256 tail).
            # ACT (the bottleneck engine) takes most groups; the otherwise-
            # idle DVE takes one 1024-col group per block, computing tanh
            # as the odd minimax poly x + x^3*(C3 + C5*x^2) (|x| < 0.8,
            # max err 1.6e-4).  Small first group starts the tanh stream
            # early; small last group keeps the tail DMA short.
            C3, C5 = -0.32618857, 0.09579417
            plan = [
                (0, 1024, "act"),
                (1024, 2048, "act"),
                (3072, 2048, "act"),
                (5120, 256, "act"),
            ]
            groups = [(b, ba, co, ki) for ba, co, ki in plan for b in range(2)]
            with tc.tile_pool(name="psum_pair", bufs=2, space="PSUM") as pq:
                for b, base, cols, kind in groups:
                    ppair = pq.tile([128, 2048], F32, tag="ppair")
                    q = 0
                    while q * 512 < cols:
                        w = min(512, cols - q * 512)
                        nc.tensor.matmul(
                            ppair[:, q * 512 : q * 512 + w],
                            lhsT=el[b],
                            rhs=rr[:, base + q * 512 : base + q * 512 + w],
                            start=True,
                            stop=True,
                        )
                        q += 1
                    ot = op_.tile([128, 2048], BF16, tag="ot")
                    if kind == "act":
                        nc.scalar.activation(
                            out=ot[:, 0:cols], in_=ppair[:, 0:cols], func=AF.Tanh
                        )
                    else:
                        xc = wp.tile([128, 1024], BF16, tag=f"xc{b}")
                        tt = wp.tile([128, 1024], BF16, tag=f"tt{b}")
                        uu = wp.tile([128, 1024], BF16, tag=f"uu{b}")
                        ww = wp.tile([128, 1024], BF16, tag=f"ww{b}")
                        nc.vector.tensor_copy(xc, ppair[:, 0:cols])
                        nc.vector.tensor_mul(tt, xc, xc)
                        nc.vector.tensor_scalar(
                            uu, tt, C5, C3,
                            op0=mybir.AluOpType.mult, op1=mybir.AluOpType.add,
                        )
                        nc.vector.tensor_mul(ww, uu, tt)
                        nc.vector.scalar_tensor_tensor(
                            ot[:, 0:cols], ww, 1.0, xc,
                            op0=mybir.AluOpType.add, op1=mybir.AluOpType.mult,
                        )
                    nc.sync.dma_start(
                        out=out_d[:, b * JFLAT + base : b * JFLAT + base + cols],
                        in_=ot[:, 0:cols],
                    )

    nc.finalize()
    return nc


def _host_prepare(inputs):
    """Gather embeddings + fold/lay out weights; returns per-core in_maps."""
    bf = ml_dtypes.bfloat16
    wi = np.asarray(inputs["word_idx"]).astype(np.int64)
    pi = np.asarray(inputs["pos_idx"]).astype(np.int64)
    ei = np.asarray(inputs["ext_idx"]).astype(np.int64)
    we = np.asarray(inputs["word_emb"], np.float32)
    pe = np.asarray(inputs["pos_emb"], np.float32)
    xe = np.asarray(inputs["ext_emb"], np.float32)
    vec = np.concatenate([we[wi], pe[pi], xe[ei]], axis=-1)  # [512, 425] f32

    w_ih_f = np.asarray(inputs["w_ih_f"], np.float32)
    w_ih_b = np.asarray(inputs["w_ih_b"], np.float32)
    b_f = np.asarray(inputs["b_f"], np.float32)
    b_b = np.asarray(inputs["b_b"], np.float32)
    w_mlp_in = np.asarray(inputs["w_mlp_in"], np.float32)
    b_mlp_in = np.asarray(inputs["b_mlp_in"], np.float32)
    w_mlp_out = np.asarray(inputs["w_mlp_out"], np.float32)
    b_mlp_out = np.asarray(inputs["b_mlp_out"], np.float32)

    # stacked gate weights+bias [426, 600]: i_f o_f i_b o_b g_f g_b
    w6 = np.zeros((426, 600), np.float32)
    cols = [
        w_ih_f[0:100], w_ih_f[300:400], w_ih_b[0:100], w_ih_b[300:400],
        w_ih_f[200:300], w_ih_b[200:300],
    ]
    biases = [
        b_f[0:100], b_f[300:400], b_b[0:100], b_b[300:400],
        b_f[200:300], b_b[200:300],
    ]
    for m, (wslab, bslab) in enumerate(zip(cols, biases)):
        w6[0:425, m * 100 : (m + 1) * 100] = wslab.T
        w6[425, m * 100 : (m + 1) * 100] = bslab

    # fold mlp_in into mlp_out: M_raw[k, r] = sum_d Win[d,k] Wout[r,d]
    m_raw = w_mlp_in.T @ w_mlp_out.T          # [625, 42]
    b_half = b_mlp_in @ w_mlp_out.T + 0.5 * b_mlp_out  # [42]

    def m_dev(hv):
        # device row order: vec (425), b' row, h_f (100), h_b (100)
        md = np.zeros((626, NREL), np.float32)
        if hv:   # cat = [h, vec]
            md[0:425] = m_raw[200:625]
            md[426:526] = m_raw[0:100]
            md[526:626] = m_raw[100:200]
        else:    # cat = [vec, h]
            md[0:425] = m_raw[0:425]
            md[426:526] = m_raw[425:525]
            md[526:626] = m_raw[525:625]
        md[425] = b_half
        return md

    m_hv, m_vh = m_dev(True), m_dev(False)

    # periodic identity block for the pairwise broadcast matmul
    ic = np.zeros((NREL, IC_PER), np.float32)
    c = np.arange(IC_PER)
    ic[c % NREL, c] = 1.0

    def fill(pk, name, arr):
        _, rows, off, width = SEG[name]
        assert arr.shape == (rows, width), (name, arr.shape, rows, width)
        pk[0:rows, off : off + width] = arr

    in_maps = []
    for core in range(8):
        ib, jq = core // 4, core % 4
        toks = np.concatenate(
            [np.arange(ib * 256, (ib + 1) * 256), np.arange(jq * 128, (jq + 1) * 128)]
        )
        vect = np.zeros((426, T), np.float32)
        vect[0:425] = vec[toks].T
        vect[425] = 1.0
        m_i = m_hv if ib == 0 else m_vh
        m_j = m_hv if jq < 2 else m_vh
        m2 = np.concatenate([m_i, m_j], axis=1)  # [626, 84]

        pk = np.zeros((128, NPK), np.float32)
        for k, (a, b) in enumerate(KS):
            fill(pk, f"vt{k}", vect[a:b])
            fill(pk, f"g6{k}", w6[a:b])
            fill(pk, f"m{k}", m2[a:b])
        fill(pk, "mh0", m2[426:526])
        fill(pk, "mh1", m2[526:626])
        fill(pk, "ic", ic)
        in_maps.append(dict(pk=pk.astype(bf)))
    return in_maps


_CACHED_NC = None


def kernel(**inputs):
    global _CACHED_NC
    in_maps = _host_prepare(inputs)
    if _CACHED_NC is None:
        _CACHED_NC = _build_program()
    res = run_bass_kernel_spmd(_CACHED_NC, in_maps, list(range(8)))
    full = np.empty((SEQ, SEQ, NREL), np.float32)
    for core in range(8):
        ib, jq = core // 4, core % 4
        blk = np.asarray(res.results[core]["out"], dtype=np.float32)
        for b in range(2):
            full[
                ib * 256 + b * 128 : ib * 256 + (b + 1) * 128,
                jq * 128 : (jq + 1) * 128,
                :,
            ] = blk[:, b * JFLAT : (b + 1) * JFLAT].reshape(128, NJ, NREL)
    return full


if __name__ == "__main__":
    rng = np.random.default_rng(0)
    demo = dict(
        word_idx=rng.integers(0, 50000, 512),
        pos_idx=rng.integers(0, 48, 512),
        ext_idx=rng.integers(0, 100000, 512),
        word_emb=rng.standard_normal((50000, 100), np.float32) * 0.05,
        pos_emb=rng.standard_normal((48, 25), np.float32) * 0.05,
        ext_emb=rng.standard_normal((100000, 300), np.float32) * 0.05,
        w_ih_f=rng.standard_normal((400, 425), np.float32) * 0.05,
        b_f=rng.standard_normal(400).astype(np.float32) * 0.05,
        w_ih_b=rng.standard_normal((400, 425), np.float32) * 0.05,
        b_b=rng.standard_normal(400).astype(np.float32) * 0.05,
        w_mlp_in=rng.standard_normal((400, 625), np.float32) * 0.05,
        b_mlp_in=rng.standard_normal(400).astype(np.float32) * 0.05,
        w_mlp_out=rng.standard_normal((42, 400), np.float32) * 0.05,
        b_mlp_out=rng.standard_normal(42).astype(np.float32) * 0.05,
    )
    out = kernel(**demo)
    print("out", out.shape, out.dtype, float(np.abs(out).max()))
